# revision 6
# baseline (speedup 1.0000x reference)
"""CoAttention Trainium2 Bass kernel, v3 (bf16, single-E, transposed-PV,
division-free softmax via LN scale-invariance).

Sharding: data-parallel over batch B=8 across 8 NeuronCores (one batch element
per core); CxC projection weights replicated.

Per-core math (all matmul operands bf16, fp32 PSUM):
  qT = w_q @ x1 + b_q  [c,l]    kT = w_k @ x2 + b_k  [c,l]
  v1 = x1^T w_v1^T     [l,c]    v2 = x2^T w_v2^T     [l,c]
    (v-biases fold into the residuals: softmax rows sum to 1)
  E[q,k] = exp(S/sqrt(C)) computed once (q-major), stored to DRAM scratch;
    the k-major orientation comes back via hardware DMA-transpose loads.
  pass A: pv[c,q] = v2^T @ E_kq (transposed PV: no PE transposes)
    LN scale-invariance: LN(pv/d + x) == LN(pv + d*x), so no softmax
    division anywhere - the denominator row d (ones-matmul over E) instead
    multiplies the residual.  z = pv + d_bc * (x1+b_v2);  out1 =
    ((z - mu)*rstd)*gamma + beta with per-q rows (mu, rstd) computed via
    ones-matmul column sums and broadcast back by rank-1 matmuls; all
    row arithmetic done on broadcast [128, 512] tiles (DVE-parallel),
    rstd via reciprocal_approx_fast (~18 bits).
  pass B: symmetric with E_qk slices reloaded, v1, x2 + b_v1.
Softmax max-subtraction skipped: logits ~ N(0,1), exp() safe in fp32.
"""

import sys

import numpy as np

try:
    import concourse.bass as bass  # noqa: F401
except ImportError:  # grading env may not have it on sys.path
    sys.path.insert(0, "/opt/trn_rl_repo")

import concourse.bass as bass
import concourse.tile as tile
from concourse import bacc, mybir
from concourse.bass_utils import run_bass_kernel_spmd

C = 512
L = 2048
B = 8
NCORES = 8
P = 128
EPS = 1e-5
INV_SQRT_C = 1.0 / float(np.sqrt(C))
F32 = mybir.dt.float32
F32R = mybir.dt.float32r
BF16 = mybir.dt.bfloat16
CT = C // P   # 4 c-tiles
LT = L // P   # 16 l-tiles
NCH = L // C  # 4 512-wide chunks

Alu = mybir.AluOpType
Act = mybir.ActivationFunctionType


def _r(ap):
    return ap.bitcast(F32R)


def _build():
    nc = bacc.Bacc(
        "TRN2",
        target_bir_lowering=False,
        debug=False,
        enable_asserts=False,
        num_devices=NCORES,
    )
    dram = lambda n, s, dt, kind: nc.dram_tensor(n, s, dt, kind=kind).ap()
    x1d = dram("x1b", [C, L], BF16, "ExternalInput")
    x2d = dram("x2b", [C, L], BF16, "ExternalInput")
    wtd = dram("wt_all", [4, C, C], BF16, "ExternalInput")
    bcolsd = dram("bcols", [P, CT, 6], F32, "ExternalInput")
    onesd = dram("ones_const", [P], F32, "ExternalInput")
    cinvd = dram("cinv_const", [P], F32, "ExternalInput")
    identd = dram("ident", [P, P], F32, "ExternalInput")
    out1d = dram("out1", [C, L], F32, "ExternalOutput")
    out2d = dram("out2", [C, L], F32, "ExternalOutput")
    e_dram = dram("e_scr", [L, L], BF16, "Internal")

    x1v = x1d.rearrange("(t p) l -> p t l", p=P)
    x2v = x2d.rearrange("(t p) l -> p t l", p=P)
    o1v = out1d.rearrange("(g p) l -> p g l", p=P)
    o2v = out2d.rearrange("(g p) l -> p g l", p=P)
    e_store_v = e_dram.rearrange("(qt p) k -> p qt k", p=P)
    e_qk_v = e_dram.rearrange("(qt p) k -> p qt k", p=P)

    with tile.TileContext(nc) as tc:
        with (
            tc.tile_pool(name="res", bufs=1) as res,
            tc.tile_pool(name="wide", bufs=2) as wide,
            tc.tile_pool(name="estg", bufs=2) as estg,
            tc.tile_pool(name="ekqp", bufs=2) as ekqp,
            tc.tile_pool(name="wts", bufs=2) as wts,
            tc.tile_pool(name="work", bufs=2) as work,
            tc.tile_pool(name="one", bufs=1) as one,
            tc.tile_pool(name="singles", bufs=1) as singles,
        ):
            # ---- constants ----
            bcols = singles.tile([P, CT, 6], F32)
            nc.sync.dma_start(out=bcols[:], in_=bcolsd)
            ones_row_f = singles.tile([1, P], F32R)
            nc.sync.dma_start(out=ones_row_f[:], in_=_r(onesd.unsqueeze(0)))
            cinv_row = singles.tile([1, P], F32R)
            nc.sync.dma_start(out=cinv_row[:], in_=_r(cinvd.unsqueeze(0)))
            ident_r = singles.tile([P, P], F32R)
            nc.sync.dma_start(out=ident_r[:], in_=_r(identd))
            da_col = singles.tile([P, LT], F32R)
            db_col = singles.tile([P, LT], F32R)
            ones_col_b = singles.tile([P, 1], BF16)
            nc.vector.memset(ones_col_b[:], 1.0)
            eps_col = singles.tile([P, 1], F32)
            nc.vector.memset(eps_col[:], EPS)

            # ---- weights first (critical path to first matmul) ----
            wT = {}
            wtv = wtd.rearrange("i (t p) c -> i p t c", p=P)
            for i, n in enumerate(("w_q", "w_k", "w_v1", "w_v2")):
                t = wts.tile([P, CT, C], BF16, tag="wt")
                nc.sync.dma_start(out=t[:], in_=wtv[i])
                wT[n] = t

            # ---- inputs as bf16 (cast during DMA), chunked for overlap ----
            x1b = res.tile([P, CT, L], BF16, tag="x1b")
            x2b = res.tile([P, CT, L], BF16, tag="x2b")
            for n in range(NCH):
                sl = slice(n * C, (n + 1) * C)
                nc.scalar.dma_start(out=x1b[:, :, sl], in_=x1v[:, :, sl])
            for n in range(NCH):
                sl = slice(n * C, (n + 1) * C)
                nc.scalar.dma_start(out=x2b[:, :, sl], in_=x2v[:, :, sl])

            # ================= projections =================
            qT = wide.tile([P, CT, L], BF16, tag="wide")
            kT = wide.tile([P, CT, L], BF16, tag="wide")
            v1 = res.tile([P, LT, C], BF16, tag="v1")
            v2 = res.tile([P, LT, C], BF16, tag="v2")

            with tc.tile_pool(name="ps_p", bufs=2, space="PSUM") as ps_p:
                for tT, xb, wn, bi in ((qT, x1b, "w_q", 0), (kT, x2b, "w_k", 1)):
                    for m in range(CT):
                        ps = ps_p.tile([P, NCH, C], F32, tag="pp")
                        for c in range(CT):
                            for n in range(NCH):
                                nc.tensor.matmul(
                                    ps[:, n, :],
                                    lhsT=wT[wn][:, c, m * P : (m + 1) * P],
                                    rhs=xb[:, c, n * C : (n + 1) * C],
                                    start=(c == 0),
                                    stop=(c == CT - 1),
                                )
                        nc.vector.tensor_scalar(
                            out=tT[:, m, :],
                            in0=ps[:].rearrange("p a b -> p (a b)"),
                            scalar1=bcols[:, m, bi : bi + 1],
                            scalar2=None,
                            op0=Alu.add,
                        )
                for vout, xb, wn in ((v2, x2b, "w_v2"), (v1, x1b, "w_v1")):
                    for g in range(4):
                        ps = ps_p.tile([P, NCH, C], F32, tag="pp")
                        for c in range(CT):
                            for j in range(4):
                                lt = g * 4 + j
                                nc.tensor.matmul(
                                    ps[:, j, :],
                                    lhsT=xb[:, c, lt * P : (lt + 1) * P],
                                    rhs=wT[wn][:, c, :],
                                    start=(c == 0),
                                    stop=(c == CT - 1),
                                )
                        nc.vector.tensor_copy(
                            out=vout[:, g * 4 : (g + 1) * 4, :], in_=ps[:]
                        )

            # residuals: x1b += b_v2, x2b += b_v1 (per-partition cols)
            for t in range(CT):
                nc.vector.tensor_scalar(
                    out=x1b[:, t, :], in0=x1b[:, t, :],
                    scalar1=bcols[:, t, 3:4], scalar2=None, op0=Alu.add)
                nc.vector.tensor_scalar(
                    out=x2b[:, t, :], in0=x2b[:, t, :],
                    scalar1=bcols[:, t, 2:3], scalar2=None, op0=Alu.add)

            # ============ E = exp(S/sqrt(C)), q-major + early tlds ============
            e_stores = []
            ekq_tiles = {}

            def tld_quarter(ci):
                t = ekqp.tile([P, LT, C], BF16, tag="ekq")
                ld = nc.sync.dma_start_transpose(
                    out=t[:], in_=e_dram[ci * C : (ci + 1) * C, :])
                from concourse.tile import add_dep_helper
                add_dep_helper(ld.ins, e_stores[ci].ins, sync=True,
                               reason="e_scr DRAM RAW (transpose)")
                ekq_tiles[ci] = t

            with tc.tile_pool(name="ps_e", bufs=2, space="PSUM") as ps_e:
                for grp in range(4):
                    est = estg.tile([P, 4, L], BF16, tag="estg")
                    for j in range(4):
                        qt = grp * 4 + j
                        ps = ps_e.tile([P, NCH, C], F32, tag="se")
                        for c in range(CT):
                            for kc in range(NCH):
                                nc.tensor.matmul(
                                    ps[:, kc, :],
                                    lhsT=qT[:, c, qt * P : (qt + 1) * P],
                                    rhs=kT[:, c, kc * C : (kc + 1) * C],
                                    start=(c == 0),
                                    stop=(c == CT - 1),
                                )
                        with nc.allow_low_precision(reason="f32r d cols"):
                            nc.scalar.activation(
                                out=est[:, j, :],
                                in_=ps[:].rearrange("p a b -> p (a b)"),
                                func=Act.Exp,
                                scale=INV_SQRT_C,
                                accum_out=da_col[:, qt : qt + 1],
                            )
                    e_stores.append(nc.sync.dma_start(
                        out=e_store_v[:, grp * 4 : (grp + 1) * 4, :], in_=est[:]
                    ))
                    if grp < 2:
                        tld_quarter(grp)

            # ================= co-attention passes =================
            def fin_front(ps_pool, esrc, xres, pv_fill, dcol, ci):
                """PV/z/zsq for one chunk; returns state for fin_back."""
                dps = ps_pool.tile([1, C], F32, tag="dr")
                if dcol is not None:
                    for j in range(4):
                        nc.tensor.matmul(
                            dps[0:1, j * P : (j + 1) * P],
                            lhsT=dcol[:, 4 * ci + j : 4 * ci + j + 1],
                            rhs=ident_r[:],
                            start=(j == 0), stop=(j == 3))
                else:
                    for kt in range(LT):
                        nc.tensor.matmul(
                            dps[0:1, :], lhsT=ones_col_b[:],
                            rhs=esrc[:, kt, :],
                            start=(kt == 0), stop=(kt == LT - 1))
                d_row = work.tile([1, C], F32R, tag="drow", bufs=1)
                nc.vector.tensor_copy(out=d_row[:], in_=dps[0:1, :])
                d_bct = ps_pool.tile([P, 2, C], F32, tag="bc2")
                d_bc = d_bct[:, 0, :]
                nc.tensor.matmul(d_bc, lhsT=ones_row_f[:], rhs=d_row[:],
                                 start=True, stop=True)
                zx = work.tile([P, CT, C], BF16, tag="zx", bufs=1)
                nc.vector.tensor_tensor(
                    out=zx[:], in0=xres,
                    in1=d_bc.unsqueeze(1).broadcast_to((P, CT, C)),
                    op=Alu.mult)
                pvt = [ps_pool.tile([P, C], F32, tag=f"pv{g}", name=f"pv{g}")
                       for g in range(CT)]
                pv_fill(pvt)
                z = work.tile([P, CT, C], BF16, tag="z")
                for g in range(CT):
                    nc.vector.tensor_tensor(out=z[:, g, :], in0=zx[:, g, :],
                                            in1=pvt[g][:], op=Alu.add)
                zsq = one.tile([P, CT, C], BF16, tag="zsq")
                nc.vector.tensor_tensor(out=zsq[:], in0=z[:], in1=z[:],
                                        op=Alu.mult)
                return z, zsq

            def fin_back(ps_pool, z, zsq, out_view):
                """stats/LN/apply/out for one chunk (runs a chunk behind)."""
                stat = ps_pool.tile([P, 2, C], F32, tag="bc2")
                for g in range(CT):
                    nc.tensor.matmul(
                        stat[0:1, 0, :], lhsT=ones_col_b[:], rhs=z[:, g, :],
                        start=(g == 0), stop=(g == CT - 1))
                for g in range(CT):
                    nc.tensor.matmul(
                        stat[0:1, 1, :], lhsT=ones_col_b[:], rhs=zsq[:, g, :],
                        start=(g == 0), stop=(g == CT - 1))
                rows_sb = work.tile([1, 2, C], F32R, tag="rows", bufs=1)
                nc.vector.tensor_copy(out=rows_sb[:], in_=stat[0:1, :, :])
                mu_bc = ps_pool.tile([P, 2, C], F32, tag="bc2")
                nc.tensor.matmul(mu_bc[:, 0, :], lhsT=cinv_row[:],
                                 rhs=rows_sb[0:1, 0, :], start=True, stop=True)
                nc.tensor.matmul(mu_bc[:, 1, :], lhsT=cinv_row[:],
                                 rhs=rows_sb[0:1, 1, :], start=True, stop=True)
                mu_sb = work.tile([P, C], F32, tag="musb", bufs=1)
                nc.vector.tensor_copy(out=mu_sb[:], in_=mu_bc[:, 0, :])
                var_sb = work.tile([P, C], F32, tag="varsb", bufs=1)
                nc.vector.tensor_tensor(out=var_sb[:], in0=mu_sb[:],
                                        in1=mu_sb[:], op=Alu.mult)
                nc.vector.tensor_tensor(out=var_sb[:], in0=mu_bc[:, 1, :],
                                        in1=var_sb[:], op=Alu.subtract)
                nc.scalar.activation(out=var_sb[:], in_=var_sb[:],
                                     func=Act.Sqrt, bias=eps_col[:], scale=1.0)
                s_bc = work.tile([P, C], F32, tag="sbc", bufs=1)
                nc.vector.reciprocal_approx_fast(out=s_bc[:], in_=var_sb[:])
                o_bc = work.tile([P, C], F32, tag="obc", bufs=1)
                nc.vector.tensor_tensor(out=o_bc[:], in0=mu_sb[:], in1=s_bc[:],
                                        op=Alu.mult)
                nc.vector.tensor_tensor(
                    out=z[:], in0=z[:],
                    in1=s_bc[:].unsqueeze(1).broadcast_to((P, CT, C)),
                    op=Alu.mult)
                nc.vector.tensor_tensor(
                    out=z[:], in0=z[:],
                    in1=o_bc[:].unsqueeze(1).broadcast_to((P, CT, C)),
                    op=Alu.subtract)
                for g in range(CT):
                    nc.vector.tensor_scalar(
                        out=z[:, g, :], in0=z[:, g, :],
                        scalar1=bcols[:, g, 4:5], scalar2=bcols[:, g, 5:6],
                        op0=Alu.mult, op1=Alu.add)
                nc.gpsimd.dma_start(out=out_view, in_=z[:])

            with tc.tile_pool(name="ps_a", bufs=1, space="PSUM") as ps_a:
                from concourse.tile import add_dep_helper
                pending = None  # (z, zsq, out_view) one chunk behind

                def make_fill_a(ekq):
                    def fill(pvt):
                        for kt in range(LT):
                            for g in range(CT):
                                nc.tensor.matmul(
                                    pvt[g][:],
                                    lhsT=v2[:, kt, g * P : (g + 1) * P],
                                    rhs=ekq[:, kt, :],
                                    start=(kt == 0), stop=(kt == LT - 1))
                    return fill

                def make_fill_b(eqk):
                    def fill(pvt):
                        for qt in range(LT):
                            for g in range(CT):
                                nc.tensor.matmul(
                                    pvt[g][:],
                                    lhsT=v1[:, qt, g * P : (g + 1) * P],
                                    rhs=eqk[:, qt, :],
                                    start=(qt == 0), stop=(qt == LT - 1))
                    return fill

                eqk_tiles = {}

                def reload_chunk(ci):
                    t = ekqp.tile([P, LT, C], BF16, tag="ekq")
                    rld = nc.sync.dma_start(
                        out=t[:], in_=e_qk_v[:, :, ci * C : (ci + 1) * C])
                    for st_ in e_stores:
                        add_dep_helper(rld.ins, st_.ins, sync=True,
                                       reason="e_scr DRAM RAW (reload)")
                    eqk_tiles[ci] = t

                for step in range(2 * NCH + 1):
                    if step < 2 * NCH:
                        # prefetch next pass-B reload one step ahead
                        pf = step - NCH + 1
                        if 0 <= pf < NCH:
                            reload_chunk(pf)
                        if step < NCH:  # pass A chunk
                            ci = step
                            if ci not in ekq_tiles:
                                tld_quarter(ci)
                            esrc = ekq_tiles[ci]
                            fill = make_fill_a(esrc)
                            xres = x1b[:, :, ci * C : (ci + 1) * C]
                            oview = o1v[:, :, ci * C : (ci + 1) * C]
                        else:  # pass B chunk
                            ci = step - NCH
                            if ci not in eqk_tiles:
                                reload_chunk(ci)
                            esrc = eqk_tiles[ci]
                            fill = make_fill_b(esrc)
                            xres = x2b[:, :, ci * C : (ci + 1) * C]
                            oview = o2v[:, :, ci * C : (ci + 1) * C]
                        dcol = da_col if step < NCH else None
                        z, zsq = fin_front(ps_a, esrc, xres, fill, dcol, ci)
                        nxt = (z, zsq, oview)
                    else:
                        nxt = None
                    if pending is not None:
                        fin_back(ps_a, *pending)
                    pending = nxt

    nc.compile()
    return nc


_NC_CACHE = {}


def _get_nc():
    if "nc" not in _NC_CACHE:
        _NC_CACHE["nc"] = _build()
    return _NC_CACHE["nc"]


def _np_bf16(a):
    """Cast fp32 array to bfloat16. Uses ml_dtypes when available, else a
    round-to-nearest-even bit trick (kept as uint16 payload)."""
    try:
        import ml_dtypes

        return a.astype(ml_dtypes.bfloat16)
    except ImportError:
        u = a.astype(np.float32).view(np.uint32)
        rounded = ((u + 0x7FFF + ((u >> 16) & 1)) >> 16).astype(np.uint16)
        return rounded.view(np.dtype([("v", np.uint16)]))["v"]


def _in_maps(inputs):
    arrs = {k: np.ascontiguousarray(np.asarray(v), dtype=np.float32)
            for k, v in inputs.items()}
    bcols = np.stack(
        [arrs[n].reshape(CT, P).T
         for n in ("b_q", "b_k", "b_v1", "b_v2", "ln_gamma", "ln_beta")],
        axis=2,
    ).astype(np.float32)
    wt_all = np.ascontiguousarray(_np_bf16(np.stack(
        [arrs[n].T for n in ("w_q", "w_k", "w_v1", "w_v2")], axis=0)))
    x1b_all = np.ascontiguousarray(_np_bf16(arrs["x1"]))
    x2b_all = np.ascontiguousarray(_np_bf16(arrs["x2"]))
    maps = []
    for b in range(NCORES):
        m = {"x1b": x1b_all[b], "x2b": x2b_all[b], "bcols": bcols,
             "wt_all": wt_all,
             "ones_const": np.ones(P, dtype=np.float32),
             "cinv_const": np.full(P, 1.0 / C, dtype=np.float32),
             "ident": np.eye(P, dtype=np.float32)}
        maps.append(m)
    return maps


def _run(inputs, trace=False):
    nc = _get_nc()
    res = run_bass_kernel_spmd(nc, _in_maps(inputs), list(range(NCORES)),
                               trace=trace)
    out1 = np.stack([r_["out1"] for r_ in res.results])
    out2 = np.stack([r_["out2"] for r_ in res.results])
    return (out1, out2), res


def kernel(**inputs):
    (out1, out2), _ = _run(inputs)
    return out1, out2
